# revision 1
# baseline (speedup 1.0000x reference)
"""Cross-Mamba selective-scan (2 branches, swapped C) on 8 Trainium2 NeuronCores.

Sharding: core c = 2*b + s handles batch b, stream s (s=0: rgb branch,
s=1: e branch).  Each core runs the full selective scan + layernorm for one
(batch, branch) pair with d_inner = 384 on-chip.

Per-core device pipeline:
  A) PE-transpose u (own stream) and xo (other stream) to d-major xT/xoT.
  B) PE matmuls: delta_raw = MT.T @ xT  (MT = (dtW @ Wxp[:R]).T, host-fused),
     B = WB.T @ xT, C = WC.T @ xoT (C from the *other* stream = cross attn).
     softplus via ACT: Ln(Exp(x + dtb) + 1).
  C) For each state idx n: broadcast B[n]/C[n] across partitions (gpsimd),
     dA = Exp(delta * A[:,n]) (ACT, per-partition scale),
     dBu = delta*u*B (DVE), hardware scan x_l = dA*x + dBu (DVE
     tensor_tensor_scan, in-place), y += xs*C (DVE).
  D) Transpose y back to l-major (PE) twice: first pass computes LN stats
     (ACT Identity/Square with accum_out), second pass normalizes, DMA out.
"""
import os
import sys

sys.path.insert(0, "/opt/trn_rl_repo")
os.environ.setdefault("MYCRO_LOCAL_CACHE", "1")

from contextlib import ExitStack

import numpy as np

B_, L, DI, N, R = 4, 2048, 384, 16, 12
P = 128
NDB = DI // P      # 3 d-blocks of 128 channels
NLT = L // P       # 16 l-tiles of 128
LC = 512           # l-chunk for matmuls / psum
NLC = L // LC      # 4
EPS = 1e-5

TRACE = False
LAST = {}

_prog = None


def _build(dumps=False, reps=1, abl=(), big_bufs=6, rep_bufs=2, stage_bufs=3, half=0, skip_gb=False):
    import concourse.bacc as bacc
    import concourse.tile as tile
    from concourse import library_config, mybir

    f32 = mybir.dt.float32
    bf16 = mybir.dt.bfloat16
    cdt = bf16 if half >= 1 else f32      # dtype of xs / crep side
    bdt = bf16 if half >= 2 else f32      # dtype of du / dBu / brep side
    AF = mybir.ActivationFunctionType
    OP = mybir.AluOpType

    nc = bacc.Bacc("TRN2", target_bir_lowering=False, debug=False, num_devices=8)

    def inp(name, shape):
        return nc.dram_tensor(name, list(shape), f32, kind="ExternalInput").ap()

    u_d = inp("u", (L, DI))
    xo_d = inp("xo", (L, DI))
    mt_d = inp("mt", (DI, DI))
    wb_d = inp("wb", (DI, N))
    wc_d = inp("wc", (DI, N))
    ar_d = inp("a_r", (P, NDB * N))
    dtb_d = inp("dtb_r", (P, NDB))
    dr_d = inp("d_r", (P, NDB))
    g_d = inp("g_rep", (P, DI))
    bb_d = inp("b_rep", (P, DI))
    id_d = inp("ident", (P, P))
    y_d = nc.dram_tensor("y", [L, DI], f32, kind="ExternalOutput").ap()
    dump_d = {}
    if dumps:
        for nm, shape in [("d_xT0", (P, L)), ("d_dsp0", (P, L)),
                          ("d_du0", (P, L)), ("d_b", (N, L)), ("d_c", (N, L)),
                          ("d_brep", (P, L)), ("d_dA0", (P, L)),
                          ("d_xs0", (P, L)), ("d_ymul0", (P, L)),
                          ("d_yacc0", (P, L)), ("d_sum", (P, NLT)),
                          ("d_mean", (P, NLT)), ("d_rstd", (P, NLT))]:
            dump_d[nm] = nc.dram_tensor(nm, list(shape), f32,
                                        kind="ExternalOutput").ap()

    with tile.TileContext(nc) as tc, ExitStack() as ctx:
        consts = ctx.enter_context(tc.tile_pool(name="consts", bufs=1))
        big = ctx.enter_context(tc.tile_pool(name="bigp", bufs=big_bufs))
        wide = ctx.enter_context(tc.tile_pool(name="widep", bufs=2))
        dsp_p = ctx.enter_context(tc.tile_pool(name="dspp", bufs=3))
        du_p = ctx.enter_context(tc.tile_pool(name="dup", bufs=3))
        rep_p = ctx.enter_context(tc.tile_pool(name="repp", bufs=rep_bufs))
        stage_p = ctx.enter_context(tc.tile_pool(name="stagep", bufs=stage_bufs))
        etmp_p = ctx.enter_context(tc.tile_pool(name="etmpp", bufs=1))
        bcst_p = ctx.enter_context(tc.tile_pool(name="bcstp", bufs=2))
        ylm_p = ctx.enter_context(tc.tile_pool(name="ylmp", bufs=2))
        dram_p = ctx.enter_context(tc.tile_pool(name="dramp", bufs=1, space="DRAM"))
        ps_big = ctx.enter_context(tc.tile_pool(name="psbig", bufs=4, space="PSUM"))
        ps_sm = ctx.enter_context(tc.tile_pool(name="pssm", bufs=2, space="PSUM"))

        # ---- constants ----
        mt_sb = consts.tile([P, NDB, DI], f32, tag="mt")
        nc.sync.dma_start(mt_sb[:], mt_d.rearrange("(k p) d -> p k d", p=P))
        wb_sb = consts.tile([P, NDB, N], f32, tag="wb")
        nc.sync.dma_start(wb_sb[:], wb_d.rearrange("(k p) n -> p k n", p=P))
        wc_sb = consts.tile([P, NDB, N], f32, tag="wc")
        nc.sync.dma_start(wc_sb[:], wc_d.rearrange("(k p) n -> p k n", p=P))
        ar_sb = consts.tile([P, NDB * N], f32, tag="ar")
        nc.sync.dma_start(ar_sb[:], ar_d[:])
        dtb_sb = consts.tile([P, NDB], f32, tag="dtb")
        nc.sync.dma_start(dtb_sb[:], dtb_d[:])
        dr_sb = consts.tile([P, NDB], f32, tag="dr")
        nc.sync.dma_start(dr_sb[:], dr_d[:])
        g_sb = consts.tile([P, DI], f32, tag="g")
        nc.sync.dma_start(g_sb[:], g_d[:])
        bb_sb = consts.tile([P, DI], f32, tag="bb")
        nc.sync.dma_start(bb_sb[:], bb_d[:])
        ident = consts.tile([P, P], f32, tag="ident")
        nc.sync.dma_start(ident[:], id_d[:])

        for _rep in range(reps):
            # ---- A: transpose inputs to d-major ----
            def load_transpose(src, dst):
                for lc in range(NLC):
                    stages = []
                    for q in range(4):
                        st = stage_p.tile([P, DI], f32, tag="stage")
                        lt = lc * 4 + q
                        nc.sync.dma_start(st[:], src[lt * P:(lt + 1) * P, :])
                        stages.append(st)
                    for db in range(NDB):
                        ps = ps_big.tile([P, LC], f32, tag="psbig")
                        for q in range(4):
                            nc.tensor.transpose(
                                ps[:, q * P:(q + 1) * P],
                                stages[q][:, db * P:(db + 1) * P],
                                ident[:],
                            )
                        nc.scalar.copy(dst[db][:, lc * LC:(lc + 1) * LC], ps[:])

            xT = [big.tile([P, L], f32, tag="big", name=f"xT{_rep}_{i}") for i in range(NDB)]
            load_transpose(u_d, xT)
            if dumps:
                nc.sync.dma_start(dump_d["d_xT0"][:], xT[0][:])
            xoT = [big.tile([P, L], f32, tag="big", name=f"xoT{_rep}_{i}") for i in range(NDB)]
            load_transpose(xo_d, xoT)

            # ---- B: projections ----
            b_dram = dram_p.tile([N, L], bdt, tag=f"bdram{_rep}", name=f"bdram{_rep}")
            c_dram = dram_p.tile([N, L], cdt, tag=f"cdram{_rep}", name=f"cdram{_rep}")
            for lc in range(NLC):
                sl = slice(lc * LC, (lc + 1) * LC)
                psb = ps_sm.tile([N, LC], f32, tag="pssm")
                for k in range(NDB):
                    nc.tensor.matmul(psb[:], wb_sb[:, k, :], xT[k][:, sl],
                                     start=(k == 0), stop=(k == NDB - 1))
                bst = bcst_p.tile([N, LC], bdt, tag="bcst")
                nc.scalar.copy(bst[:], psb[:])
                nc.sync.dma_start(b_dram[:, sl], bst[:])
                psc = ps_sm.tile([N, LC], f32, tag="pssm")
                for k in range(NDB):
                    nc.tensor.matmul(psc[:], wc_sb[:, k, :], xoT[k][:, sl],
                                     start=(k == 0), stop=(k == NDB - 1))
                cst = bcst_p.tile([N, LC], cdt, tag="cbcst")
                nc.scalar.copy(cst[:], psc[:])
                nc.sync.dma_start(c_dram[:, sl], cst[:])

            dsps, dus, yaccs = [], [], []
            for db in range(NDB):
                dsp = dsp_p.tile([P, L], f32, tag="dsp")
                for lc in range(NLC):
                    sl = slice(lc * LC, (lc + 1) * LC)
                    ps = ps_big.tile([P, LC], f32, tag="psbig")
                    for k in range(NDB):
                        nc.tensor.matmul(ps[:], mt_sb[:, k, db * P:(db + 1) * P],
                                         xT[k][:, sl],
                                         start=(k == 0), stop=(k == NDB - 1))
                    # softplus(x + dtb) = Ln(Exp(x + dtb) + 1)
                    et = etmp_p.tile([P, LC], f32, tag="etmp")
                    nc.scalar.activation(et[:], ps[:], AF.Exp,
                                         bias=dtb_sb[:, db:db + 1], scale=1.0)
                    nc.scalar.activation(dsp[:, sl], et[:], AF.Ln, bias=1.0, scale=1.0)
                dsps.append(dsp)
                du = du_p.tile([P, L], bdt, tag="du")
                nc.vector.tensor_mul(du[:], dsp[:], xT[db][:])
                dus.append(du)
                if dumps and db == 0:
                    nc.sync.dma_start(dump_d["d_dsp0"][:], dsp[:])
                    nc.sync.dma_start(dump_d["d_du0"][:], du[:])
                if db == 0:
                    yacc_all = wide.tile([P, NDB * L], f32, tag="wide",
                                         name=f"yacc_all{_rep}")
                nc.scalar.activation(yacc_all[:, db * L:(db + 1) * L],
                                     xT[db][:], AF.Copy,
                                     scale=dr_sb[:, db:db + 1])
                yaccs.append(yacc_all[:, db * L:(db + 1) * L])

            # ---- C: scan over state dim ----
            if dumps:
                nc.sync.dma_start(dump_d["d_b"][:], b_dram[:])
                nc.sync.dma_start(dump_d["d_c"][:], c_dram[:])
            for n in range(N):
                if "nobcast" not in abl:
                    brep = rep_p.tile([P, L], bdt, tag="brep")
                    nc.sync.dma_start(brep[:],
                                      b_dram[n:n + 1, :].partition_broadcast(P))
                    crep = rep_p.tile([P, L], cdt, tag="crep")
                    nc.sync.dma_start(crep[:],
                                      c_dram[n:n + 1, :].partition_broadcast(P))
                else:
                    brep, crep = dus[0], dus[1]
                if dumps and n == 0:
                    nc.sync.dma_start(dump_d["d_brep"][:], brep[:])
                xs_all = wide.tile([P, NDB * L], f32, tag="wide",
                                   name=f"xsall{_rep}_{n}")
                for db in range(NDB):
                    if "noact" in abl:
                        xs = big.tile([P, L], f32, tag="big", name="xs_na")
                        if "nodbu" not in abl:
                            dBu = big.tile([P, L], f32, tag="big", name="dbu_na")
                            nc.vector.tensor_mul(dBu[:], dus[db][:], brep[:])
                        else:
                            dBu = dus[db]
                        nc.vector.tensor_tensor_scan(xs[:], dsps[db][:], dBu[:],
                                                     0.0, OP.mult, OP.add)
                        nc.vector.tensor_mul(xs[:], xs[:], crep[:])
                        nc.vector.tensor_add(yaccs[db][:], yaccs[db][:], xs[:])
                        continue
                    dA = big.tile([P, L], f32, tag="big")
                    if "noexp" not in abl:
                        nc.scalar.activation(dA[:], dsps[db][:], AF.Exp,
                                             scale=ar_sb[:, db * N + n: db * N + n + 1])
                    else:
                        nc.scalar.copy(dA[:], dsps[db][:])
                    if dumps and n == 0 and db == 0:
                        nc.sync.dma_start(dump_d["d_dA0"][:], dA[:])
                    if "nodbu" not in abl:
                        dBu = big.tile([P, L], bdt, tag="dbub" if half >= 2 else "big",
                                       bufs=2 if half >= 2 else big_bufs)
                        nc.vector.tensor_mul(dBu[:], dus[db][:], brep[:])
                    else:
                        dBu = dus[db]
                    xs = xs_all[:, db * L:(db + 1) * L]
                    if "noscan" not in abl:
                        nc.vector.tensor_tensor_scan(xs[:], dA[:], dBu[:], 0.0,
                                                     OP.mult, OP.add)
                    else:
                        nc.vector.tensor_mul(xs[:], dA[:], dBu[:])
                    if dumps and n == 0 and db == 0:
                        nc.sync.dma_start(dump_d["d_xs0"][:], xs[:])
                    if "noymul" not in abl:
                        nc.vector.tensor_mul(xs[:], xs[:], crep[:])
                    if dumps and n == 0 and db == 0:
                        nc.sync.dma_start(dump_d["d_ymul0"][:], xs[:])
                if "noyacc" not in abl:
                    nc.vector.tensor_add(yacc_all[:], yacc_all[:], xs_all[:])

            # ---- D: transpose to l-major twice: stats pass, then normalize ----
            if dumps:
                nc.sync.dma_start(dump_d["d_yacc0"][:], yaccs[0][:])

            def transpose_ltile(lt):
                psy = ps_big.tile([P, DI], f32, tag="psbig", name=f"psy{_rep}_{lt}")
                for db in range(NDB):
                    nc.tensor.transpose(psy[:, db * P:(db + 1) * P],
                                        yaccs[db][:, lt * P:(lt + 1) * P], ident[:])
                ylm = ylm_p.tile([P, DI], f32, tag="ylm", name=f"ylm{_rep}_{lt}")
                nc.scalar.copy(ylm[:], psy[:])
                return ylm

            sum_c = consts.tile([P, NLT], f32, tag="sumc")
            ssq_c = consts.tile([P, NLT], f32, tag="ssqc")
            for lt in range(NLT):
                ylm = transpose_ltile(lt)
                dum = ylm_p.tile([P, DI], f32, tag="sqs", bufs=1)
                nc.scalar.activation(dum[:], ylm[:], AF.Identity, scale=1.0,
                                     accum_out=sum_c[:, lt:lt + 1])
                nc.scalar.activation(dum[:], ylm[:], AF.Square, scale=1.0,
                                     accum_out=ssq_c[:, lt:lt + 1])
            mean = consts.tile([P, NLT], f32, tag="mean")
            rstd = consts.tile([P, NLT], f32, tag="rstd")
            msq = consts.tile([P, NLT], f32, tag="msq")
            var = consts.tile([P, NLT], f32, tag="var")
            nc.vector.tensor_scalar_mul(mean[:], sum_c[:], 1.0 / DI)
            nc.vector.tensor_scalar_mul(msq[:], ssq_c[:], 1.0 / DI)
            # var = msq - mean^2 ; rstd = exp(-0.5*ln(var + eps))
            nc.vector.tensor_mul(var[:], mean[:], mean[:])
            nc.vector.tensor_sub(var[:], msq[:], var[:])
            nc.vector.tensor_scalar_add(var[:], var[:], EPS)
            nc.scalar.activation(var[:], var[:], AF.Ln, scale=1.0)
            nc.scalar.activation(rstd[:], var[:], AF.Exp, scale=-0.5)
            nmr = consts.tile([P, NLT], f32, tag="nmr")
            nc.vector.tensor_mul(nmr[:], mean[:], rstd[:])
            nc.vector.tensor_scalar_mul(nmr[:], nmr[:], -1.0)
            if dumps:
                nc.sync.dma_start(dump_d["d_sum"][:], sum_c[:])
                nc.sync.dma_start(dump_d["d_mean"][:], mean[:])
                nc.sync.dma_start(dump_d["d_rstd"][:], rstd[:])

            for lt in range(NLT):
                ylm = transpose_ltile(lt)
                nc.scalar.activation(ylm[:], ylm[:], AF.Identity,
                                     scale=rstd[:, lt:lt + 1],
                                     bias=nmr[:, lt:lt + 1])
                if not skip_gb:
                    nc.vector.tensor_mul(ylm[:], ylm[:], g_sb[:])
                    nc.vector.tensor_add(ylm[:], ylm[:], bb_sb[:])
                nc.sync.dma_start(y_d[lt * P:(lt + 1) * P, :], ylm[:])

    nc.compile()
    return nc


_progs = {}


def _get_prog(skip_gb=False):
    if skip_gb not in _progs:
        _progs[skip_gb] = _build(skip_gb=skip_gb)
    return _progs[skip_gb]


def _host_prep(Wxp, dtW, dtb, A_log, D, g, b, Wxp_other):
    """Per-branch constant tensors (host-side weight fusion / relayout)."""
    Wxp = np.asarray(Wxp, np.float32)
    Wxp_other = np.asarray(Wxp_other, np.float32)
    dtW = np.asarray(dtW, np.float32)
    mt = (dtW @ Wxp[:R]).T.copy()                     # (DI, DI) [d', d]
    wb = Wxp[R:R + N].T.copy()                        # (DI, N)
    wc = Wxp_other[R + N:R + 2 * N].T.copy()          # (DI, N)  cross-C
    A_neg = -np.exp(np.asarray(A_log, np.float32))    # (DI, N)
    a_r = A_neg.reshape(NDB, P, N).transpose(1, 0, 2).reshape(P, NDB * N).copy()
    dtb_r = np.asarray(dtb, np.float32).reshape(NDB, P).T.copy()
    d_r = np.asarray(D, np.float32).reshape(NDB, P).T.copy()
    g_rep = np.broadcast_to(np.asarray(g, np.float32), (P, DI)).copy()
    b_rep = np.broadcast_to(np.asarray(b, np.float32), (P, DI)).copy()
    return dict(mt=mt, wb=wb, wc=wc, a_r=a_r, dtb_r=dtb_r, d_r=d_r,
                g_rep=g_rep, b_rep=b_rep,
                ident=np.eye(P, dtype=np.float32))


def kernel(x_rgb, x_e, Wxp1, Wxp2, dtW1, dtW2, dtb1, dtb2,
           A_log1, A_log2, D1, D2, g1, b1, g2, b2):
    from concourse.bass_utils import run_bass_kernel_spmd

    # When the LN affine params are identity (as in the reference init),
    # dispatch to a build that skips the two affine passes.
    skip_gb = all(
        np.allclose(np.asarray(a, np.float32), v)
        for a, v in ((g1, 1.0), (g2, 1.0), (b1, 0.0), (b2, 0.0)))
    nc = _get_prog(skip_gb)
    x_rgb = np.asarray(x_rgb, np.float32)
    x_e = np.asarray(x_e, np.float32)
    w1 = _host_prep(Wxp1, dtW1, dtb1, A_log1, D1, g1, b1, Wxp2)
    w2 = _host_prep(Wxp2, dtW2, dtb2, A_log2, D2, g2, b2, Wxp1)

    in_maps = []
    for b in range(B_):
        in_maps.append(dict(u=x_rgb[b], xo=x_e[b], **w1))
        in_maps.append(dict(u=x_e[b], xo=x_rgb[b], **w2))

    res = run_bass_kernel_spmd(nc, in_maps, list(range(8)), trace=TRACE)
    LAST["res"] = res
    ys = [res.results[i]["y"] for i in range(8)]
    out_rgb = np.stack([ys[2 * b] for b in range(B_)])
    out_e = np.stack([ys[2 * b + 1] for b in range(B_)])
    return out_rgb, out_e



# revision 4
# speedup vs baseline: 4.5470x; 4.5470x over previous
"""Cross-Mamba selective-scan (2 branches, swapped C) on 8 Trainium2 NeuronCores.

Sharding: core c = 2*b + s handles batch b, stream s (s=0: rgb branch,
s=1: e branch).  Each core runs the full selective scan + layernorm for one
(batch, branch) pair with d_inner = 384 on-chip.

v2 pipeline (per core), engine-balanced:
  A) PE-transpose u (own stream) and xo (other stream) to d-major xT/xoT.
  B) PE matmuls: delta_raw = MT.T @ xT, B = WB.T @ xT, C = WC.T @ xoT
     (cross-attention C comes from the other stream), softplus on ACT,
     du = dsp*xT (Pool), Du = D*xT (ACT), B/C staged to DRAM in bf16.
  C) n-loop (16 states): brep/crep bf16 row-broadcasts via DMA;
     per d-block: dA=exp(A*dsp) on ACT (f32), dBu=du*brep (DVE bf16),
     hardware scan x=dA*x+dBu on DVE (f32 state), ym=xs*crep,
     y-accumulation: d-blocks 0,1 via PE identity-matmul into PSUM (f32),
     d-block 2 via Pool tensor-add into SBUF f32.
  D) PSUM->SBUF copies (ACT), then two PE transpose passes for layernorm:
     stats via ACT accum_out, then normalize + DMA out.
"""
import os
import sys

sys.path.insert(0, "/opt/trn_rl_repo")
os.environ.setdefault("MYCRO_LOCAL_CACHE", "1")

from contextlib import ExitStack

import numpy as np

B_, L, DI, N, R = 4, 2048, 384, 16, 12
P = 128
NDB = DI // P      # 3 d-blocks of 128 channels
NLT = L // P       # 16 l-tiles of 128
LC = 512           # l-chunk for matmuls / psum
NLC = L // LC      # 4
EPS = 1e-5

TRACE = False
LAST = {}

# default engine/dtype config; override via _build(cfg=...)
CFG = dict(
    dbu_bf=True,        # dBu mul in bf16 (DVE fast path); False -> f32
    xs_bf=False,        # scan output dtype bf16; False -> f32
    ym_pool=(2,),       # d-blocks whose ym mul runs on Pool (rest: DVE)
    dbu_pool=(),        # d-blocks whose dBu mul runs on Pool
    acc_pool=(2,),      # d-blocks accumulated via Pool adds (rest: PE psum)
    du_pool=True,       # du mul on Pool
)


def _build(dumps=False, reps=1, cfg=None, skip_gb=False):
    import concourse.bacc as bacc
    import concourse.tile as tile
    from concourse import mybir

    c = dict(CFG)
    if cfg:
        c.update(cfg)
    f32 = mybir.dt.float32
    bf16 = mybir.dt.bfloat16
    dbu_dt = bf16 if c["dbu_bf"] else f32
    xs_dt = bf16 if c["xs_bf"] else f32
    AF = mybir.ActivationFunctionType
    OP = mybir.AluOpType

    pe_acc_dbs = tuple(db for db in range(NDB) if db not in c["acc_pool"])
    assert len(pe_acc_dbs) * NLC <= 8, "psum accumulation exceeds 8 banks"

    nc = bacc.Bacc("TRN2", target_bir_lowering=False, debug=False, num_devices=8)

    def inp(name, shape):
        return nc.dram_tensor(name, list(shape), f32, kind="ExternalInput").ap()

    u_d = inp("u", (L, DI))
    xo_d = inp("xo", (L, DI))
    mt_d = inp("mt", (DI, DI))
    wb_d = inp("wb", (DI, N))
    wc_d = inp("wc", (DI, N))
    ar_d = inp("a_r", (P, NDB * N))
    dtb_d = inp("dtb_r", (P, NDB))
    dr_d = inp("d_r", (P, NDB))
    g_d = inp("g_rep", (P, DI))
    bb_d = inp("b_rep", (P, DI))
    id_d = inp("ident", (P, P))
    y_d = nc.dram_tensor("y", [L, DI], f32, kind="ExternalOutput").ap()

    with tile.TileContext(nc) as tc, ExitStack() as ctx:
        consts = ctx.enter_context(tc.tile_pool(name="consts", bufs=1))
        bigf = ctx.enter_context(tc.tile_pool(name="bigf", bufs=6))
        dsp_p = ctx.enter_context(tc.tile_pool(name="dspp", bufs=3))
        du_p = ctx.enter_context(tc.tile_pool(name="dup", bufs=3))
        acc_p = ctx.enter_context(tc.tile_pool(name="accp", bufs=1))
        rep_p = ctx.enter_context(tc.tile_pool(name="repp", bufs=2))
        bfw_p = ctx.enter_context(tc.tile_pool(name="bfwp", bufs=1))
        stage_p = ctx.enter_context(tc.tile_pool(name="stagep", bufs=4))
        etmp_p = ctx.enter_context(tc.tile_pool(name="etmpp", bufs=2))
        bcst_p = ctx.enter_context(tc.tile_pool(name="bcstp", bufs=2))
        ylm_p = ctx.enter_context(tc.tile_pool(name="ylmp", bufs=3))
        dram_p = ctx.enter_context(tc.tile_pool(name="dramp", bufs=1, space="DRAM"))
        ps_p = ctx.enter_context(tc.tile_pool(name="psp", bufs=8, space="PSUM"))

        # ---- constants ----
        mt_sb = consts.tile([P, NDB, DI], f32, tag="mt", name="mt_sb")
        nc.sync.dma_start(mt_sb[:], mt_d.rearrange("(k p) d -> p k d", p=P))
        wb_sb = consts.tile([P, NDB, N], f32, tag="wb", name="wb_sb")
        nc.sync.dma_start(wb_sb[:], wb_d.rearrange("(k p) n -> p k n", p=P))
        wc_sb = consts.tile([P, NDB, N], f32, tag="wc", name="wc_sb")
        nc.sync.dma_start(wc_sb[:], wc_d.rearrange("(k p) n -> p k n", p=P))
        ar_sb = consts.tile([P, NDB * N], f32, tag="ar", name="ar_sb")
        nc.sync.dma_start(ar_sb[:], ar_d[:])
        dtb_sb = consts.tile([P, NDB], f32, tag="dtb", name="dtb_sb")
        nc.sync.dma_start(dtb_sb[:], dtb_d[:])
        dr_sb = consts.tile([P, NDB], f32, tag="dr", name="dr_sb")
        nc.sync.dma_start(dr_sb[:], dr_d[:])
        g_sb = consts.tile([P, DI], f32, tag="g", name="g_sb")
        nc.sync.dma_start(g_sb[:], g_d[:])
        bb_sb = consts.tile([P, DI], f32, tag="bb", name="bb_sb")
        nc.sync.dma_start(bb_sb[:], bb_d[:])
        ident = consts.tile([P, P], f32, tag="ident", name="ident")
        nc.sync.dma_start(ident[:], id_d[:])
        identb = consts.tile([P, P], bf16, tag="identb", name="identb")
        nc.vector.tensor_copy(identb[:], ident[:])

        for _rep in range(reps):
            # ---- A: transpose inputs to d-major ----
            def load_transpose(src, dst_tag, rep_i):
                outs = []
                for lc in range(NLC):
                    stages = []
                    for q in range(4):
                        st = stage_p.tile([P, DI], f32, tag="stage", name="st")
                        lt = lc * 4 + q
                        nc.sync.dma_start(st[:], src[lt * P:(lt + 1) * P, :])
                        stages.append(st)
                    for db in range(NDB):
                        if lc == 0:
                            t = bigf.tile([P, L], f32, tag="big",
                                          name=f"{dst_tag}{rep_i}_{db}")
                            outs.append(t)
                        ps = ps_p.tile([P, LC], f32, tag="ps", name="psA")
                        for q in range(4):
                            nc.tensor.transpose(
                                ps[:, q * P:(q + 1) * P],
                                stages[q][:, db * P:(db + 1) * P],
                                ident[:],
                            )
                        nc.scalar.copy(outs[db][:, lc * LC:(lc + 1) * LC], ps[:])
                return outs

            xT = load_transpose(u_d, "xT", _rep)
            xoT = load_transpose(xo_d, "xoT", _rep)

            # ---- B: projections ----
            b_dram = dram_p.tile([N, L], bf16, tag=f"bdram{_rep}", name=f"bdram{_rep}")
            c_dram = dram_p.tile([N, L], bf16, tag=f"cdram{_rep}", name=f"cdram{_rep}")
            for lc in range(NLC):
                sl = slice(lc * LC, (lc + 1) * LC)
                psb = ps_p.tile([N, LC], f32, tag="ps", name="psB")
                for k in range(NDB):
                    nc.tensor.matmul(psb[:], wb_sb[:, k, :], xT[k][:, sl],
                                     start=(k == 0), stop=(k == NDB - 1))
                bst = bcst_p.tile([N, LC], bf16, tag="bcst", name="bst")
                nc.scalar.copy(bst[:], psb[:])
                nc.sync.dma_start(b_dram[:, sl], bst[:])
                psc = ps_p.tile([N, LC], f32, tag="ps", name="psC")
                for k in range(NDB):
                    nc.tensor.matmul(psc[:], wc_sb[:, k, :], xoT[k][:, sl],
                                     start=(k == 0), stop=(k == NDB - 1))
                cst = bcst_p.tile([N, LC], bf16, tag="cbcst", name="cst")
                nc.scalar.copy(cst[:], psc[:])
                nc.sync.dma_start(c_dram[:, sl], cst[:])

            dsps, dus = [], []
            for db in range(NDB):
                dsp = dsp_p.tile([P, L], f32, tag="dsp", name="dsp")
                for lc in range(NLC):
                    sl = slice(lc * LC, (lc + 1) * LC)
                    ps = ps_p.tile([P, LC], f32, tag="ps", name="psD")
                    for k in range(NDB):
                        nc.tensor.matmul(ps[:], mt_sb[:, k, db * P:(db + 1) * P],
                                         xT[k][:, sl],
                                         start=(k == 0), stop=(k == NDB - 1))
                    # softplus(x + dtb) = Ln(Exp(x + dtb) + 1)
                    et = etmp_p.tile([P, LC], f32, tag="etmp", name="et")
                    nc.scalar.activation(et[:], ps[:], AF.Exp,
                                         bias=dtb_sb[:, db:db + 1], scale=1.0)
                    nc.scalar.activation(dsp[:, sl], et[:], AF.Ln, bias=1.0, scale=1.0)
                dsps.append(dsp)
                du = du_p.tile([P, L], dbu_dt, tag="du", name="du")
                if c["du_pool"]:
                    nc.gpsimd.tensor_mul(du[:], dsp[:], xT[db][:])
                else:
                    nc.vector.tensor_mul(du[:], dsp[:], xT[db][:])
                dus.append(du)

            # y init: Du = D*xT.  PE-acc d-blocks get a bf16 copy matmul'd in
            # with start=True; pool d-blocks get f32 yacc tile init.
            yaccs = {}
            psaccs = {}
            for db in range(NDB):
                if db in pe_acc_dbs:
                    dub = bfw_p.tile([P, L], bf16, tag="dub", name=f"dub{db}", bufs=1)
                    nc.scalar.activation(dub[:], xT[db][:], AF.Copy,
                                         scale=dr_sb[:, db:db + 1])
                    banks = []
                    for lc in range(NLC):
                        pb = ps_p.tile([P, LC], f32, tag="ps",
                                       name=f"psacc{_rep}_{db}_{lc}")
                        nc.tensor.matmul(pb[:], identb[:],
                                         dub[:, lc * LC:(lc + 1) * LC],
                                         start=True, stop=False,
                                         skip_group_check=True)
                        banks.append(pb)
                    psaccs[db] = banks
                else:
                    yacc = acc_p.tile([P, L], f32, tag=f"yacc{db}", name=f"yacc{db}")
                    nc.scalar.activation(yacc[:], xT[db][:], AF.Copy,
                                         scale=dr_sb[:, db:db + 1])
                    yaccs[db] = yacc

            # ---- C: scan over state dim ----
            for n in range(N):
                brep = rep_p.tile([P, L], bf16, tag="brep", name="brep")
                nc.sync.dma_start(brep[:],
                                  b_dram[n:n + 1, :].partition_broadcast(P))
                crep = rep_p.tile([P, L], bf16, tag="crep", name="crep")
                nc.sync.dma_start(crep[:],
                                  c_dram[n:n + 1, :].partition_broadcast(P))
                for db in range(NDB):
                    dA = bigf.tile([P, L], f32, tag="big", name="dA")
                    nc.scalar.activation(dA[:], dsps[db][:], AF.Exp,
                                         scale=ar_sb[:, db * N + n: db * N + n + 1])
                    dBu = bfw_p.tile([P, L], dbu_dt, tag="dbu", name="dBu", bufs=2)
                    if db in c["dbu_pool"]:
                        nc.gpsimd.tensor_mul(dBu[:], dus[db][:], brep[:])
                    else:
                        nc.vector.tensor_mul(dBu[:], dus[db][:], brep[:])
                    xs = bfw_p.tile([P, L], xs_dt, tag="xs", name="xs", bufs=2)
                    nc.vector.tensor_tensor_scan(xs[:], dA[:], dBu[:], 0.0,
                                                 OP.mult, OP.add)
                    ym = bfw_p.tile([P, L], bf16, tag="ym", name="ym", bufs=3)
                    if db in c["ym_pool"]:
                        nc.gpsimd.tensor_mul(ym[:], xs[:], crep[:])
                    else:
                        nc.vector.tensor_mul(ym[:], xs[:], crep[:])
                    if db in pe_acc_dbs:
                        for lc in range(NLC):
                            nc.tensor.matmul(psaccs[db][lc][:], identb[:],
                                             ym[:, lc * LC:(lc + 1) * LC],
                                             start=False, stop=(n == N - 1),
                                             skip_group_check=True)
                    else:
                        nc.gpsimd.tensor_add(yaccs[db][:], yaccs[db][:], ym[:])

            # psum -> sbuf for PE-accumulated d-blocks
            for db in pe_acc_dbs:
                yacc = acc_p.tile([P, L], f32, tag=f"yacc{db}", name=f"yaccp{db}")
                for lc in range(NLC):
                    nc.scalar.copy(yacc[:, lc * LC:(lc + 1) * LC],
                                   psaccs[db][lc][:])
                yaccs[db] = yacc

            # ---- D: transpose to l-major twice: stats pass, then normalize ----
            def transpose_ltile(lt, nm):
                psy = ps_p.tile([P, DI], f32, tag="ps", name=f"psy{nm}")
                for db in range(NDB):
                    nc.tensor.transpose(psy[:, db * P:(db + 1) * P],
                                        yaccs[db][:, lt * P:(lt + 1) * P], ident[:])
                ylm = ylm_p.tile([P, DI], f32, tag="ylm", name=f"ylm{nm}")
                nc.scalar.copy(ylm[:], psy[:])
                return ylm

            sum_c = consts.tile([P, NLT], f32, tag="sumc", name="sum_c")
            ssq_c = consts.tile([P, NLT], f32, tag="ssqc", name="ssq_c")
            for lt in range(NLT):
                ylm = transpose_ltile(lt, "s")
                dum = ylm_p.tile([P, DI], f32, tag="sqs", name="dum", bufs=1)
                nc.scalar.activation(dum[:], ylm[:], AF.Identity, scale=1.0,
                                     accum_out=sum_c[:, lt:lt + 1])
                nc.scalar.activation(dum[:], ylm[:], AF.Square, scale=1.0,
                                     accum_out=ssq_c[:, lt:lt + 1])
            mean = consts.tile([P, NLT], f32, tag="mean", name="mean")
            rstd = consts.tile([P, NLT], f32, tag="rstd", name="rstd")
            msq = consts.tile([P, NLT], f32, tag="msq", name="msq")
            var = consts.tile([P, NLT], f32, tag="var", name="var")
            nc.vector.tensor_scalar_mul(mean[:], sum_c[:], 1.0 / DI)
            nc.vector.tensor_scalar_mul(msq[:], ssq_c[:], 1.0 / DI)
            # var = msq - mean^2 ; rstd = exp(-0.5*ln(var + eps))
            nc.vector.tensor_mul(var[:], mean[:], mean[:])
            nc.vector.tensor_sub(var[:], msq[:], var[:])
            nc.vector.tensor_scalar_add(var[:], var[:], EPS)
            nc.scalar.activation(var[:], var[:], AF.Ln, scale=1.0)
            nc.scalar.activation(rstd[:], var[:], AF.Exp, scale=-0.5)
            nmr = consts.tile([P, NLT], f32, tag="nmr", name="nmr")
            nc.vector.tensor_mul(nmr[:], mean[:], rstd[:])
            nc.vector.tensor_scalar_mul(nmr[:], nmr[:], -1.0)

            for lt in range(NLT):
                ylm = transpose_ltile(lt, "n")
                nc.scalar.activation(ylm[:], ylm[:], AF.Identity,
                                     scale=rstd[:, lt:lt + 1],
                                     bias=nmr[:, lt:lt + 1])
                if not skip_gb:
                    nc.vector.tensor_mul(ylm[:], ylm[:], g_sb[:])
                    nc.vector.tensor_add(ylm[:], ylm[:], bb_sb[:])
                nc.sync.dma_start(y_d[lt * P:(lt + 1) * P, :], ylm[:])

    nc.compile()
    return nc


_progs = {}


def _get_prog(skip_gb=False):
    if skip_gb not in _progs:
        _progs[skip_gb] = _build(skip_gb=skip_gb)
    return _progs[skip_gb]


def _host_prep(Wxp, dtW, dtb, A_log, D, g, b, Wxp_other):
    """Per-branch constant tensors (host-side weight fusion / relayout)."""
    Wxp = np.asarray(Wxp, np.float32)
    Wxp_other = np.asarray(Wxp_other, np.float32)
    dtW = np.asarray(dtW, np.float32)
    mt = (dtW @ Wxp[:R]).T.copy()                     # (DI, DI) [d', d]
    wb = Wxp[R:R + N].T.copy()                        # (DI, N)
    wc = Wxp_other[R + N:R + 2 * N].T.copy()          # (DI, N)  cross-C
    A_neg = -np.exp(np.asarray(A_log, np.float32))    # (DI, N)
    a_r = A_neg.reshape(NDB, P, N).transpose(1, 0, 2).reshape(P, NDB * N).copy()
    dtb_r = np.asarray(dtb, np.float32).reshape(NDB, P).T.copy()
    d_r = np.asarray(D, np.float32).reshape(NDB, P).T.copy()
    g_rep = np.broadcast_to(np.asarray(g, np.float32), (P, DI)).copy()
    b_rep = np.broadcast_to(np.asarray(b, np.float32), (P, DI)).copy()
    return dict(mt=mt, wb=wb, wc=wc, a_r=a_r, dtb_r=dtb_r, d_r=d_r,
                g_rep=g_rep, b_rep=b_rep,
                ident=np.eye(P, dtype=np.float32))


def kernel(x_rgb, x_e, Wxp1, Wxp2, dtW1, dtW2, dtb1, dtb2,
           A_log1, A_log2, D1, D2, g1, b1, g2, b2):
    from concourse.bass_utils import run_bass_kernel_spmd

    # When the LN affine params are identity (as in the reference init),
    # dispatch to a build that skips the two affine passes.
    skip_gb = all(
        np.allclose(np.asarray(a, np.float32), v)
        for a, v in ((g1, 1.0), (g2, 1.0), (b1, 0.0), (b2, 0.0)))
    nc = _get_prog(skip_gb)
    x_rgb = np.asarray(x_rgb, np.float32)
    x_e = np.asarray(x_e, np.float32)
    w1 = _host_prep(Wxp1, dtW1, dtb1, A_log1, D1, g1, b1, Wxp2)
    w2 = _host_prep(Wxp2, dtW2, dtb2, A_log2, D2, g2, b2, Wxp1)

    in_maps = []
    for b in range(B_):
        in_maps.append(dict(u=x_rgb[b], xo=x_e[b], **w1))
        in_maps.append(dict(u=x_e[b], xo=x_rgb[b], **w2))

    res = run_bass_kernel_spmd(nc, in_maps, list(range(8)), trace=TRACE)
    LAST["res"] = res
    ys = [res.results[i]["y"] for i in range(8)]
    out_rgb = np.stack([ys[2 * b] for b in range(B_)])
    out_e = np.stack([ys[2 * b + 1] for b in range(B_)])
    return out_rgb, out_e


# revision 10
# speedup vs baseline: 4.7426x; 1.0430x over previous
"""Cross-Mamba selective-scan (2 branches, swapped C) on 8 Trainium2 NeuronCores.

Sharding: core c = 2*b + s handles batch b, stream s (s=0: rgb branch,
s=1: e branch).  Each core runs the full selective scan + layernorm for one
(batch, branch) pair with d_inner = 384 on-chip.

v2 pipeline (per core), engine-balanced:
  A) PE-transpose u (own stream) and xo (other stream) to d-major xT/xoT.
  B) PE matmuls: delta_raw = MT.T @ xT, B = WB.T @ xT, C = WC.T @ xoT
     (cross-attention C comes from the other stream), softplus on ACT,
     du = dsp*xT (Pool), Du = D*xT (ACT), B/C staged to DRAM in bf16.
  C) n-loop (16 states): brep/crep bf16 row-broadcasts via DMA;
     per d-block: dA=exp(A*dsp) on ACT (f32), dBu=du*brep (DVE bf16),
     hardware scan x=dA*x+dBu on DVE (f32 state), ym=xs*crep,
     y-accumulation: d-blocks 0,1 via PE identity-matmul into PSUM (f32),
     d-block 2 via Pool tensor-add into SBUF f32.
  D) PSUM->SBUF copies (ACT), then two PE transpose passes for layernorm:
     stats via ACT accum_out, then normalize + DMA out.
"""
import os
import sys

sys.path.insert(0, "/opt/trn_rl_repo")
os.environ.setdefault("MYCRO_LOCAL_CACHE", "1")

from contextlib import ExitStack

import numpy as np

B_, L, DI, N, R = 4, 2048, 384, 16, 12
P = 128
NDB = DI // P      # 3 d-blocks of 128 channels
NLT = L // P       # 16 l-tiles of 128
LC = 512           # l-chunk for matmuls / psum
NLC = L // LC      # 4
EPS = 1e-5

TRACE = False
LAST = {}

# default engine/dtype config; override via _build(cfg=...)
CFG = dict(
    dbu_bf=True,        # dBu mul in bf16 (DVE fast path); False -> f32
    xs_bf=False,        # scan output dtype bf16; False -> f32
    ym_pool=(2,),       # d-blocks whose ym mul runs on Pool (rest: DVE)
    dbu_pool=(1,),      # d-blocks whose dBu mul runs on Pool
    acc_pool=(2,),      # d-blocks accumulated via Pool adds (rest: PE psum)
    du_pool=True,       # du mul on Pool
)


def _build(dumps=False, reps=1, cfg=None, skip_gb=False):
    import concourse.bacc as bacc
    import concourse.tile as tile
    from concourse import mybir

    c = dict(CFG)
    if cfg:
        c.update(cfg)
    f32 = mybir.dt.float32
    bf16 = mybir.dt.bfloat16
    dbu_dt = bf16 if c["dbu_bf"] else f32
    xs_dt = bf16 if c["xs_bf"] else f32
    AF = mybir.ActivationFunctionType
    OP = mybir.AluOpType

    pe_acc_dbs = tuple(db for db in range(NDB) if db not in c["acc_pool"])
    assert len(pe_acc_dbs) * NLC <= 8, "psum accumulation exceeds 8 banks"

    nc = bacc.Bacc("TRN2", target_bir_lowering=False, debug=False, num_devices=8)

    def inp(name, shape):
        return nc.dram_tensor(name, list(shape), f32, kind="ExternalInput").ap()

    u_d = inp("u", (L, DI))
    xo_d = inp("xo", (L, DI))
    mt_d = inp("mt", (DI, DI))
    wb_d = inp("wb", (DI, N))
    wc_d = inp("wc", (DI, N))
    ar_d = inp("a_r", (P, NDB * N))
    dtb_d = inp("dtb_r", (P, NDB))
    dr_d = inp("d_r", (P, NDB))
    g_d = inp("g_rep", (P, DI))
    bb_d = inp("b_rep", (P, DI))
    id_d = inp("ident", (P, P))
    y_d = nc.dram_tensor("y", [L, DI], f32, kind="ExternalOutput").ap()

    with tile.TileContext(nc) as tc, ExitStack() as ctx:
        consts = ctx.enter_context(tc.tile_pool(name="consts", bufs=1))
        bigf = ctx.enter_context(tc.tile_pool(name="bigf", bufs=3))
        bigh = ctx.enter_context(tc.tile_pool(name="bigh", bufs=6))
        dsp_p = ctx.enter_context(tc.tile_pool(name="dspp", bufs=3))
        du_p = ctx.enter_context(tc.tile_pool(name="dup", bufs=3))
        acc_p = ctx.enter_context(tc.tile_pool(name="accp", bufs=1))
        rep_p = ctx.enter_context(tc.tile_pool(name="repp", bufs=2))
        bfw_p = ctx.enter_context(tc.tile_pool(name="bfwp", bufs=1))
        stage_p = ctx.enter_context(tc.tile_pool(name="stagep", bufs=4))
        etmp_p = ctx.enter_context(tc.tile_pool(name="etmpp", bufs=2))
        bcst_p = ctx.enter_context(tc.tile_pool(name="bcstp", bufs=2))
        ylm_p = ctx.enter_context(tc.tile_pool(name="ylmp", bufs=3))
        dram_p = ctx.enter_context(tc.tile_pool(name="dramp", bufs=1, space="DRAM"))
        ps_p = ctx.enter_context(tc.tile_pool(name="psp", bufs=8, space="PSUM"))

        # ---- constants ----
        mt_sb = consts.tile([P, NDB, DI], f32, tag="mt", name="mt_sb")
        nc.sync.dma_start(mt_sb[:], mt_d.rearrange("(k p) d -> p k d", p=P))
        wb_sb = consts.tile([P, NDB, N], f32, tag="wb", name="wb_sb")
        nc.sync.dma_start(wb_sb[:], wb_d.rearrange("(k p) n -> p k n", p=P))
        wc_sb = consts.tile([P, NDB, N], f32, tag="wc", name="wc_sb")
        nc.sync.dma_start(wc_sb[:], wc_d.rearrange("(k p) n -> p k n", p=P))
        ar_sb = consts.tile([P, NDB * N], f32, tag="ar", name="ar_sb")
        nc.sync.dma_start(ar_sb[:], ar_d[:])
        dtb_sb = consts.tile([P, NDB], f32, tag="dtb", name="dtb_sb")
        nc.sync.dma_start(dtb_sb[:], dtb_d[:])
        dr_sb = consts.tile([P, NDB], f32, tag="dr", name="dr_sb")
        nc.sync.dma_start(dr_sb[:], dr_d[:])
        g_sb = consts.tile([P, DI], f32, tag="g", name="g_sb")
        nc.sync.dma_start(g_sb[:], g_d[:])
        bb_sb = consts.tile([P, DI], f32, tag="bb", name="bb_sb")
        nc.sync.dma_start(bb_sb[:], bb_d[:])
        ident = consts.tile([P, P], f32, tag="ident", name="ident")
        nc.sync.dma_start(ident[:], id_d[:])
        identb = consts.tile([P, P], bf16, tag="identb", name="identb")
        nc.vector.tensor_copy(identb[:], ident[:])
        mtb = consts.tile([P, NDB, DI], bf16, tag="mtb", name="mtb")
        nc.vector.tensor_copy(mtb[:], mt_sb[:])
        wbb = consts.tile([P, NDB, N], bf16, tag="wbb", name="wbb")
        nc.vector.tensor_copy(wbb[:], wb_sb[:])
        wcb = consts.tile([P, NDB, N], bf16, tag="wcb", name="wcb")
        nc.vector.tensor_copy(wcb[:], wc_sb[:])

        for _rep in range(reps):
            # ---- A+B fused: load u in bf16, transpose, project delta per l-chunk ----
            b_dram = dram_p.tile([N, L], bf16, tag=f"bdram{_rep}", name=f"bdram{_rep}")
            c_dram = dram_p.tile([N, L], bf16, tag=f"cdram{_rep}", name=f"cdram{_rep}")

            xT = [bigh.tile([P, L], bf16, tag="bigh", name=f"xT{_rep}_{i}")
                  for i in range(NDB)]
            dsps = [dsp_p.tile([P, L], f32, tag="dsp", name=f"dsp{_rep}_{i}")
                    for i in range(NDB)]
            for lc in range(NLC):
                sl = slice(lc * LC, (lc + 1) * LC)
                stages = []
                for q in range(4):
                    st = stage_p.tile([P, DI], bf16, tag="stage", name="st")
                    lt = lc * 4 + q
                    nc.gpsimd.dma_start(st[:], u_d[lt * P:(lt + 1) * P, :])
                    stages.append(st)
                for db in range(NDB):
                    ps = ps_p.tile([P, LC], bf16, tag="ps", name="psA")
                    for q in range(4):
                        nc.tensor.transpose(
                            ps[:, q * P:(q + 1) * P],
                            stages[q][:, db * P:(db + 1) * P],
                            identb[:],
                        )
                    nc.scalar.copy(xT[db][:, sl], ps[:])
                for db in range(NDB):
                    psd = ps_p.tile([P, LC], f32, tag="ps", name="psD")
                    for k in range(NDB):
                        nc.tensor.matmul(psd[:], mtb[:, k, db * P:(db + 1) * P],
                                         xT[k][:, sl],
                                         start=(k == 0), stop=(k == NDB - 1))
                    # softplus(x + dtb) = Ln(Exp(x + dtb) + 1)
                    et = etmp_p.tile([P, LC], f32, tag="etmp", name="et")
                    nc.scalar.activation(et[:], psd[:], AF.Exp,
                                         bias=dtb_sb[:, db:db + 1], scale=1.0)
                    nc.scalar.activation(dsps[db][:, sl], et[:], AF.Ln, bias=1.0,
                                         scale=1.0)

            # B projection (needs only xT)
            for lc in range(NLC):
                sl = slice(lc * LC, (lc + 1) * LC)
                psb = ps_p.tile([N, LC], f32, tag="ps", name="psB")
                for k in range(NDB):
                    nc.tensor.matmul(psb[:], wbb[:, k, :], xT[k][:, sl],
                                     start=(k == 0), stop=(k == NDB - 1))
                bst = bcst_p.tile([N, LC], bf16, tag="bcst", name="bst")
                nc.scalar.copy(bst[:], psb[:])
                nc.sync.dma_start(b_dram[:, sl], bst[:])

            # xo: load, transpose, C projection per l-chunk
            xoT = [bigh.tile([P, L], bf16, tag="bigh", name=f"xoT{_rep}_{i}")
                   for i in range(NDB)]
            for lc in range(NLC):
                sl = slice(lc * LC, (lc + 1) * LC)
                stages = []
                for q in range(4):
                    st = stage_p.tile([P, DI], bf16, tag="stage", name="st")
                    lt = lc * 4 + q
                    nc.gpsimd.dma_start(st[:], xo_d[lt * P:(lt + 1) * P, :])
                    stages.append(st)
                for db in range(NDB):
                    ps = ps_p.tile([P, LC], bf16, tag="ps", name="psA2")
                    for q in range(4):
                        nc.tensor.transpose(
                            ps[:, q * P:(q + 1) * P],
                            stages[q][:, db * P:(db + 1) * P],
                            identb[:],
                        )
                    nc.scalar.copy(xoT[db][:, sl], ps[:])
                psc = ps_p.tile([N, LC], f32, tag="ps", name="psC")
                for k in range(NDB):
                    nc.tensor.matmul(psc[:], wcb[:, k, :], xoT[k][:, sl],
                                     start=(k == 0), stop=(k == NDB - 1))
                cst = bcst_p.tile([N, LC], bf16, tag="cbcst", name="cst")
                nc.scalar.copy(cst[:], psc[:])
                nc.sync.dma_start(c_dram[:, sl], cst[:])

            dus = []
            for db in range(NDB):
                du = du_p.tile([P, L], dbu_dt, tag="du", name="du")
                if c["du_pool"]:
                    nc.gpsimd.tensor_mul(du[:], dsps[db][:], xT[db][:])
                else:
                    nc.vector.tensor_mul(du[:], dsps[db][:], xT[db][:])
                dus.append(du)

            # y init: Du = D*xT.  PE-acc d-blocks get a bf16 copy matmul'd in
            # with start=True; pool d-blocks get f32 yacc tile init.
            yaccs = {}
            psaccs = {}
            for db in range(NDB):
                if db in pe_acc_dbs:
                    dub = bfw_p.tile([P, L], bf16, tag="dub", name=f"dub{db}", bufs=1)
                    nc.scalar.activation(dub[:], xT[db][:], AF.Copy,
                                         scale=dr_sb[:, db:db + 1])
                    banks = []
                    for lc in range(NLC):
                        pb = ps_p.tile([P, LC], f32, tag="ps",
                                       name=f"psacc{_rep}_{db}_{lc}")
                        nc.tensor.matmul(pb[:], identb[:],
                                         dub[:, lc * LC:(lc + 1) * LC],
                                         start=True, stop=False,
                                         skip_group_check=True)
                        banks.append(pb)
                    psaccs[db] = banks
                else:
                    yacc = acc_p.tile([P, L], f32, tag=f"yacc{db}", name=f"yacc{db}")
                    nc.scalar.activation(yacc[:], xT[db][:], AF.Copy,
                                         scale=dr_sb[:, db:db + 1])
                    yaccs[db] = yacc

            # ---- C: scan over state dim ----
            for n in range(N):
                brep = rep_p.tile([P, L], bf16, tag="brep", name="brep")
                nc.sync.dma_start(brep[:],
                                  b_dram[n:n + 1, :].partition_broadcast(P))
                crep = rep_p.tile([P, L], bf16, tag="crep", name="crep")
                nc.sync.dma_start(crep[:],
                                  c_dram[n:n + 1, :].partition_broadcast(P))
                for db in range(NDB):
                    dA = bigf.tile([P, L], f32, tag="big", name="dA")
                    nc.scalar.activation(dA[:], dsps[db][:], AF.Exp,
                                         scale=ar_sb[:, db * N + n: db * N + n + 1])
                    dBu = bfw_p.tile([P, L], dbu_dt, tag="dbu", name="dBu", bufs=2)
                    if db in c["dbu_pool"]:
                        nc.gpsimd.tensor_mul(dBu[:], dus[db][:], brep[:])
                    else:
                        nc.vector.tensor_mul(dBu[:], dus[db][:], brep[:])
                    xs = bfw_p.tile([P, L], xs_dt, tag="xs", name="xs", bufs=2)
                    nc.vector.tensor_tensor_scan(xs[:], dA[:], dBu[:], 0.0,
                                                 OP.mult, OP.add)
                    ym = bfw_p.tile([P, L], bf16, tag="ym", name="ym", bufs=3)
                    if db in c["ym_pool"]:
                        nc.gpsimd.tensor_mul(ym[:], xs[:], crep[:])
                    else:
                        nc.vector.tensor_mul(ym[:], xs[:], crep[:])
                    if db in pe_acc_dbs:
                        for lc in range(NLC):
                            nc.tensor.matmul(psaccs[db][lc][:], identb[:],
                                             ym[:, lc * LC:(lc + 1) * LC],
                                             start=False, stop=(n == N - 1),
                                             skip_group_check=True)
                    else:
                        nc.gpsimd.tensor_add(yaccs[db][:], yaccs[db][:], ym[:])

            # psum -> sbuf for PE-accumulated d-blocks
            for db in pe_acc_dbs:
                yacc = acc_p.tile([P, L], f32, tag=f"yacc{db}", name=f"yaccp{db}")
                for lc in range(NLC):
                    nc.scalar.copy(yacc[:, lc * LC:(lc + 1) * LC],
                                   psaccs[db][lc][:])
                yaccs[db] = yacc

            # ---- D: one transpose pass; stats + normalize from resident bf16 tiles ----
            sum_c = consts.tile([P, NLT], f32, tag="sumc", name="sum_c")
            ssq_c = consts.tile([P, NLT], f32, tag="ssqc", name="ssq_c")
            ylmbs = []
            for lt in range(NLT):
                psy = ps_p.tile([P, DI], f32, tag="ps", name=f"psy{lt}")
                for db in range(NDB):
                    nc.tensor.transpose(psy[:, db * P:(db + 1) * P],
                                        yaccs[db][:, lt * P:(lt + 1) * P], ident[:])
                ylmb = ylm_p.tile([P, DI], bf16, tag="ylmb", name=f"ylmb{lt}",
                                  bufs=NLT)
                nc.scalar.activation(ylmb[:], psy[:], AF.Identity, scale=1.0,
                                     accum_out=sum_c[:, lt:lt + 1])
                dum = ylm_p.tile([P, DI], f32, tag="sqs", name="dum", bufs=1)
                nc.scalar.activation(dum[:], psy[:], AF.Square, scale=1.0,
                                     accum_out=ssq_c[:, lt:lt + 1])
                ylmbs.append(ylmb)
            mean = consts.tile([P, NLT], f32, tag="mean", name="mean")
            rstd = consts.tile([P, NLT], f32, tag="rstd", name="rstd")
            msq = consts.tile([P, NLT], f32, tag="msq", name="msq")
            var = consts.tile([P, NLT], f32, tag="var", name="var")
            nc.vector.tensor_scalar_mul(mean[:], sum_c[:], 1.0 / DI)
            nc.vector.tensor_scalar_mul(msq[:], ssq_c[:], 1.0 / DI)
            # var = msq - mean^2 ; rstd = exp(-0.5*ln(var + eps))
            nc.vector.tensor_mul(var[:], mean[:], mean[:])
            nc.vector.tensor_sub(var[:], msq[:], var[:])
            nc.vector.tensor_scalar_add(var[:], var[:], EPS)
            nc.scalar.activation(var[:], var[:], AF.Ln, scale=1.0)
            nc.scalar.activation(rstd[:], var[:], AF.Exp, scale=-0.5)
            nmr = consts.tile([P, NLT], f32, tag="nmr", name="nmr")
            nc.vector.tensor_mul(nmr[:], mean[:], rstd[:])
            nc.vector.tensor_scalar_mul(nmr[:], nmr[:], -1.0)

            for lt in range(NLT):
                ylo = ylm_p.tile([P, DI], f32, tag="ylo", name="ylo", bufs=2)
                nc.scalar.activation(ylo[:], ylmbs[lt][:], AF.Identity,
                                     scale=rstd[:, lt:lt + 1],
                                     bias=nmr[:, lt:lt + 1])
                if not skip_gb:
                    nc.vector.tensor_mul(ylo[:], ylo[:], g_sb[:])
                    nc.vector.tensor_add(ylo[:], ylo[:], bb_sb[:])
                nc.sync.dma_start(y_d[lt * P:(lt + 1) * P, :], ylo[:])

    nc.compile()
    return nc


_progs = {}


def _get_prog(skip_gb=False):
    if skip_gb not in _progs:
        _progs[skip_gb] = _build(skip_gb=skip_gb)
    return _progs[skip_gb]


def _host_prep(Wxp, dtW, dtb, A_log, D, g, b, Wxp_other):
    """Per-branch constant tensors (host-side weight fusion / relayout)."""
    Wxp = np.asarray(Wxp, np.float32)
    Wxp_other = np.asarray(Wxp_other, np.float32)
    dtW = np.asarray(dtW, np.float32)
    mt = (dtW @ Wxp[:R]).T.copy()                     # (DI, DI) [d', d]
    wb = Wxp[R:R + N].T.copy()                        # (DI, N)
    wc = Wxp_other[R + N:R + 2 * N].T.copy()          # (DI, N)  cross-C
    A_neg = -np.exp(np.asarray(A_log, np.float32))    # (DI, N)
    a_r = A_neg.reshape(NDB, P, N).transpose(1, 0, 2).reshape(P, NDB * N).copy()
    dtb_r = np.asarray(dtb, np.float32).reshape(NDB, P).T.copy()
    d_r = np.asarray(D, np.float32).reshape(NDB, P).T.copy()
    g_rep = np.broadcast_to(np.asarray(g, np.float32), (P, DI)).copy()
    b_rep = np.broadcast_to(np.asarray(b, np.float32), (P, DI)).copy()
    return dict(mt=mt, wb=wb, wc=wc, a_r=a_r, dtb_r=dtb_r, d_r=d_r,
                g_rep=g_rep, b_rep=b_rep,
                ident=np.eye(P, dtype=np.float32))


def kernel(x_rgb, x_e, Wxp1, Wxp2, dtW1, dtW2, dtb1, dtb2,
           A_log1, A_log2, D1, D2, g1, b1, g2, b2):
    from concourse.bass_utils import run_bass_kernel_spmd

    # When the LN affine params are identity (as in the reference init),
    # dispatch to a build that skips the two affine passes.
    skip_gb = all(
        np.allclose(np.asarray(a, np.float32), v)
        for a, v in ((g1, 1.0), (g2, 1.0), (b1, 0.0), (b2, 0.0)))
    nc = _get_prog(skip_gb)
    x_rgb = np.asarray(x_rgb, np.float32)
    x_e = np.asarray(x_e, np.float32)
    w1 = _host_prep(Wxp1, dtW1, dtb1, A_log1, D1, g1, b1, Wxp2)
    w2 = _host_prep(Wxp2, dtW2, dtb2, A_log2, D2, g2, b2, Wxp1)

    in_maps = []
    for b in range(B_):
        in_maps.append(dict(u=x_rgb[b], xo=x_e[b], **w1))
        in_maps.append(dict(u=x_e[b], xo=x_rgb[b], **w2))

    res = run_bass_kernel_spmd(nc, in_maps, list(range(8)), trace=TRACE)
    LAST["res"] = res
    ys = [res.results[i]["y"] for i in range(8)]
    out_rgb = np.stack([ys[2 * b] for b in range(B_)])
    out_e = np.stack([ys[2 * b + 1] for b in range(B_)])
    return out_rgb, out_e
